# revision 1
# baseline (speedup 1.0000x reference)
"""Trainium2 Bass kernel for nn_Attention_v2_cross (dense transformer, 8 cores).

Sharding: 8 cores = 4 batches x 2 query-halves (data parallel over batch and
query positions). Every core holds the full weights and the full context for
its batch, so attention needs no cross-device communication; the kv projection
is duplicated across the two cores sharing a batch (+25% matmul flops, zero
collectives).

Per-core pipeline (all matmuls in fp32r = single-pass FP22, full PE rate at
free-dim >= 256):
  1. LN(x) folded into the q projection: stats per row via ones-matmul on the
     transposed activations, apply as x*a+b with a,b broadcast across
     partitions via K=1 matmuls.
  2. q/k projected output-transposed ([e, row]); v projected row-major
     ([row, e]) - that orientation split is what the sim and attn@v matmuls
     need, and both projections cost the same.
  3. Per head: sim = qT.T @ kT (K=64 per r, accumulated over r=12), row-max,
     exp((sim-max)*128), row-sum, normalize, PE-transpose the normalized P
     tiles, then attn@v with v as the stationary operand (M=64).
  4. Out projection from the attention output (staged transposed in DRAM),
     then the final layernorm row-major.
"""

import os
import numpy as np

B, N, R, C = 4, 1024, 12, 512
H, D = 8, 64
E = H * D            # 512
NQ = N // 2          # 512 queries per core
NKJ = N              # 1024 keys per core
ALPHA = 128.0
EPS = 1e-5
XCOLS = R * NQ       # 6144  (col = r*NQ + i)
CCOLS = R * NKJ      # 12288 (col = r*NKJ + j)
P = 128

_CACHE = {}


def _build_program():
    from contextlib import ExitStack
    import concourse.bass as bass
    import concourse.tile as tile
    from concourse import bacc
    from concourse import mybir
    from concourse.masks import make_identity

    F32 = mybir.dt.float32
    F32R = mybir.dt.float32r
    AF = mybir.ActivationFunctionType
    AX = mybir.AxisListType.X

    nc = bacc.Bacc("TRN2", target_bir_lowering=False, debug=False, num_devices=8)

    xT = nc.dram_tensor("xT", [C, XCOLS], F32R, kind="ExternalInput").ap()
    ctxT = nc.dram_tensor("ctxT", [C, CCOLS], F32R, kind="ExternalInput").ap()
    wqT = nc.dram_tensor("wqT", [C, E], F32R, kind="ExternalInput").ap()
    wkT = nc.dram_tensor("wkT", [C, E], F32R, kind="ExternalInput").ap()
    wvT = nc.dram_tensor("wvT", [C, E], F32R, kind="ExternalInput").ap()
    woT = nc.dram_tensor("woT", [E, C], F32R, kind="ExternalInput").ap()
    nullk = nc.dram_tensor("nullk", [D, 2], F32R, kind="ExternalInput").ap()
    onesc = nc.dram_tensor("onesc", [P, 1], F32R, kind="ExternalInput").ap()
    nullv = nc.dram_tensor("nullv", [1, D], F32R, kind="ExternalInput").ap()
    outg = nc.dram_tensor("outg", [1, C], F32, kind="ExternalInput").ap()
    out = nc.dram_tensor("out", [XCOLS, C], F32, kind="ExternalOutput").ap()

    with ExitStack() as ctx:
        tc = ctx.enter_context(tile.TileContext(nc))

        const = ctx.enter_context(tc.tile_pool(name="const", bufs=1))
        dram = ctx.enter_context(tc.tile_pool(name="dram", bufs=1, space="DRAM"))

        identity = const.tile([P, P], F32)
        make_identity(nc, identity[:])
        ones_col = const.tile([P, 1], F32R)
        nc.sync.dma_start(ones_col[:], onesc[:, :])
        ones_row = const.tile([1, P], F32)
        nc.vector.memset(ones_row[:], 1.0)
        nullk_s = const.tile([P, 2], F32R)
        nc.sync.dma_start(nullk_s[0:D, :], nullk[:, :])
        nc.sync.dma_start(nullk_s[D : 2 * D, :], nullk[:, :])
        nullv_s = const.tile([1, D], F32R)
        nc.sync.dma_start(nullv_s[:, :], nullv[:, :])
        outg_s = const.tile([P, C], F32)
        nc.sync.dma_start(outg_s[:, :], outg.to_broadcast((P, C)))
        eps_1 = const.tile([1, 1], F32)
        nc.vector.memset(eps_1[:], EPS)
        eps_P = const.tile([P, 1], F32)
        nc.vector.memset(eps_P[:], EPS)

        qT_d = dram.tile([P, 4, XCOLS], F32R)    # qT[e, col]: e = ec*128+p
        kT_d = dram.tile([P, 4, CCOLS], F32R)
        vM_d = dram.tile([P, CCOLS // P, E], F32R)  # v rows (r,j): row = rc*128+p
        aoT_d = dram.tile([P, 4, XCOLS], F32R)

        # ---------------- Stage 1: projections -------------------------
        with tc.tile_pool(name="w1", bufs=1) as wpool, \
             tc.tile_pool(name="s1", bufs=8) as s1, \
             tc.tile_pool(name="s1b", bufs=4) as s1b, \
             tc.tile_pool(name="p1", bufs=2, space="PSUM") as p1, \
             tc.tile_pool(name="p1s", bufs=1, space="PSUM") as p1s:

            wq_s = wpool.tile([P, 4, E], F32R)
            wk_s = wpool.tile([P, 4, E], F32R)
            wv_s = wpool.tile([P, 4, E], F32R)
            for cc in range(4):
                nc.sync.dma_start(wq_s[:, cc, :], wqT[cc * P : (cc + 1) * P, :])
                nc.sync.dma_start(wk_s[:, cc, :], wkT[cc * P : (cc + 1) * P, :])
                nc.sync.dma_start(wv_s[:, cc, :], wvT[cc * P : (cc + 1) * P, :])

            # ---- 1a: LN(x) + q projection (transposed out) ----
            for rb in range(R):
                xts = []
                for cc in range(4):
                    t = s1.tile([P, NQ], F32R, tag="xt")
                    nc.sync.dma_start(t[:], xT[cc * P : (cc + 1) * P, rb * NQ : (rb + 1) * NQ])
                    xts.append(t)
                psum_sum = p1s.tile([1, NQ], F32, tag="stat_sum")
                psum_sq = p1s.tile([1, NQ], F32, tag="stat_sq")
                for cc in range(4):
                    nc.tensor.matmul(psum_sum[:], ones_col[:].bitcast(F32R),
                                     xts[cc][:].bitcast(F32R),
                                     start=(cc == 0), stop=(cc == 3))
                sqs = []
                for cc in range(4):
                    sq = s1.tile([P, NQ], F32R, tag="sq")
                    nc.scalar.activation(sq[:], xts[cc][:].bitcast(F32), AF.Square)
                    sqs.append(sq)
                for cc in range(4):
                    nc.tensor.matmul(psum_sq[:], ones_col[:].bitcast(F32R),
                                     sqs[cc][:].bitcast(F32R),
                                     start=(cc == 0), stop=(cc == 3))
                mean = s1b.tile([1, NQ], F32, tag="mean")
                nc.scalar.mul(mean[:], psum_sum[:], 1.0 / C)
                msq = s1b.tile([1, NQ], F32, tag="msq")
                nc.scalar.activation(msq[:], mean[:], AF.Square)
                var = s1b.tile([1, NQ], F32, tag="var")
                nc.scalar.mul(var[:], psum_sq[:], 1.0 / C)
                nc.vector.tensor_sub(var[:], var[:], msq[:])
                std = s1b.tile([1, NQ], F32, tag="std")
                nc.scalar.activation(std[:], var[:], AF.Sqrt, bias=eps_1[:])
                inv = s1b.tile([1, NQ], F32, tag="inv")
                nc.vector.reciprocal(inv[:], std[:])
                negb = s1b.tile([1, NQ], F32, tag="negb")
                nc.vector.tensor_mul(negb[:], mean[:], inv[:])
                nc.scalar.mul(negb[:], negb[:], -1.0)
                # broadcast a (=inv) and b (=-mean*inv) across partitions via K=1 matmul
                a_b = p1s.tile([P, NQ], F32, tag="a_b")
                nc.tensor.matmul(a_b[:], ones_row[:], inv[:], start=True, stop=True)
                b_b = p1s.tile([P, NQ], F32, tag="b_b")
                nc.tensor.matmul(b_b[:], ones_row[:], negb[:], start=True, stop=True)
                xns = []
                for cc in range(4):
                    xn = s1.tile([P, NQ], F32R, tag="xn")
                    nc.vector.tensor_mul(xn[:], xts[cc][:].bitcast(F32), a_b[:])
                    nc.vector.tensor_add(xn[:], xn[:].bitcast(F32), b_b[:])
                    xns.append(xn)
                for ec in range(4):
                    pq = p1.tile([P, NQ], F32, tag="proj")
                    for cc in range(4):
                        nc.tensor.matmul(
                            pq[:],
                            wq_s[:, cc, ec * P : (ec + 1) * P].bitcast(F32R),
                            xns[cc][:].bitcast(F32R),
                            start=(cc == 0), stop=(cc == 3))
                    qs = s1b.tile([P, NQ], F32R, tag="qstage")
                    nc.any.tensor_copy(qs[:], pq[:])
                    nc.sync.dma_start(qT_d[:, ec, rb * NQ : (rb + 1) * NQ], qs[:])

            # ---- 1b: k projection (transposed) + v projection (row-major) ----
            for cb in range(CCOLS // NQ):  # 24 blocks of 512 context columns
                cts = []
                for cc in range(4):
                    t = s1.tile([P, NQ], F32R, tag="ct")
                    nc.sync.dma_start(t[:], ctxT[cc * P : (cc + 1) * P, cb * NQ : (cb + 1) * NQ])
                    cts.append(t)
                for ec in range(4):
                    pk = p1.tile([P, NQ], F32, tag="proj")
                    for cc in range(4):
                        nc.tensor.matmul(
                            pk[:],
                            wk_s[:, cc, ec * P : (ec + 1) * P].bitcast(F32R),
                            cts[cc][:].bitcast(F32R),
                            start=(cc == 0), stop=(cc == 3))
                    ks = s1b.tile([P, NQ], F32R, tag="kstage")
                    nc.any.tensor_copy(ks[:], pk[:])
                    nc.sync.dma_start(kT_d[:, ec, cb * NQ : (cb + 1) * NQ], ks[:])
                for rc4 in range(4):
                    pv = p1.tile([P, E], F32, tag="proj")
                    for cc in range(4):
                        nc.tensor.matmul(
                            pv[:],
                            cts[cc][:, rc4 * P : (rc4 + 1) * P].bitcast(F32R),
                            wv_s[:, cc, :].bitcast(F32R),
                            start=(cc == 0), stop=(cc == 3))
                    vs = s1b.tile([P, E], F32R, tag="vstage")
                    nc.any.tensor_copy(vs[:], pv[:])
                    nc.sync.dma_start(vM_d[:, cb * 4 + rc4, :], vs[:])

        # ---------------- Stage 2: attention ---------------------------
        with tc.tile_pool(name="kq2", bufs=1) as kq2, \
             tc.tile_pool(name="pt2", bufs=2) as pt2, \
             tc.tile_pool(name="s2", bufs=4) as s2, \
             tc.tile_pool(name="st2", bufs=6) as st2, \
             tc.tile_pool(name="v2", bufs=6) as v2, \
             tc.tile_pool(name="pa2", bufs=1, space="PSUM") as pa2, \
             tc.tile_pool(name="pb2", bufs=1, space="PSUM") as pb2, \
             tc.tile_pool(name="pc2", bufs=2, space="PSUM") as pc2:

            JC = NKJ // P  # 8 key chunks of 128
            for g in range(4):  # head pairs
                kpair = kq2.tile([P, CCOLS], F32R, tag="kpair")
                nc.sync.dma_start(kpair[:], kT_d[:, g, :])
                qpair = kq2.tile([P, XCOLS], F32R, tag="qpair")
                nc.sync.dma_start(qpair[:], qT_d[:, g, :])
                for hh in range(2):
                    h = 2 * g + hh
                    pb = hh * D  # partition base: 0 or 64
                    PT = pt2.tile([P, JC, NQ], F32R, tag="PT")
                    PnT = pt2.tile([1, NQ], F32R, tag="PnT")
                    for ib in range(NQ // P):  # 4 query blocks
                        ps = []
                        for jb in range(2):
                            pj = pa2.tile([P, NQ], F32, tag=f"sim{jb}")
                            for r in range(R):
                                nc.tensor.matmul(
                                    pj[:],
                                    qpair[pb : pb + D, r * NQ + ib * P : r * NQ + (ib + 1) * P].bitcast(F32R),
                                    kpair[pb : pb + D, r * NKJ + jb * NQ : r * NKJ + (jb + 1) * NQ].bitcast(F32R),
                                    start=(r == 0), stop=(r == R - 1))
                            ps.append(pj)
                        pn = pb2.tile([P, 2], F32, tag="simnull")
                        for r in range(R):
                            nc.tensor.matmul(
                                pn[:],
                                qpair[pb : pb + D, r * NQ + ib * P : r * NQ + (ib + 1) * P].bitcast(F32R),
                                nullk_s[pb : pb + D, :].bitcast(F32R),
                                start=(r == 0), stop=(r == R - 1))
                        m = st2.tile([P, 1], F32, tag="m")
                        m1 = st2.tile([P, 1], F32, tag="m1")
                        nc.vector.reduce_max(m[:], ps[0][:], axis=AX)
                        nc.vector.reduce_max(m1[:], ps[1][:], axis=AX)
                        nc.vector.tensor_max(m[:], m[:], m1[:])
                        nc.vector.tensor_max(m[:], m[:], pn[:, 0:1])
                        negm = st2.tile([P, 1], F32, tag="negm")
                        nc.scalar.mul(negm[:], m[:], -ALPHA)
                        e0 = s2.tile([P, NQ], F32, tag="e0")
                        e1 = s2.tile([P, NQ], F32, tag="e1")
                        nc.scalar.activation(e0[:], ps[0][:], AF.Exp, bias=negm[:], scale=ALPHA)
                        nc.scalar.activation(e1[:], ps[1][:], AF.Exp, bias=negm[:], scale=ALPHA)
                        en = st2.tile([P, 1], F32, tag="en")
                        nc.scalar.activation(en[:], pn[:, 0:1], AF.Exp, bias=negm[:], scale=ALPHA)
                        s0 = st2.tile([P, 1], F32, tag="s0")
                        s1r = st2.tile([P, 1], F32, tag="s1r")
                        nc.vector.reduce_sum(s0[:], e0[:], axis=AX)
                        nc.vector.reduce_sum(s1r[:], e1[:], axis=AX)
                        den = st2.tile([P, 1], F32, tag="den")
                        nc.vector.tensor_add(den[:], s0[:], s1r[:])
                        nc.vector.tensor_add(den[:], den[:], en[:])
                        dinv = st2.tile([P, 1], F32, tag="dinv")
                        nc.vector.reciprocal(dinv[:], den[:])
                        nc.vector.tensor_mul(e0[:], e0[:], dinv[:].to_broadcast((P, NQ)))
                        nc.vector.tensor_mul(e1[:], e1[:], dinv[:].to_broadcast((P, NQ)))
                        pnorm = st2.tile([P, 1], F32, tag="pnorm")
                        nc.vector.tensor_mul(pnorm[:], en[:], dinv[:])
                        for jb in range(2):
                            src = e0 if jb == 0 else e1
                            for c4 in range(4):
                                tp = pc2.tile([P, P], F32, tag="tp")
                                nc.tensor.transpose(tp[:], src[:, c4 * P : (c4 + 1) * P], identity[:])
                                nc.any.tensor_copy(PT[:, jb * 4 + c4, ib * P : (ib + 1) * P], tp[:])
                        tpn = pb2.tile([1, P], F32, tag="tpn")
                        nc.tensor.transpose(tpn[:], pnorm[:, :], identity[:])
                        nc.any.tensor_copy(PnT[:, ib * P : (ib + 1) * P], tpn[:])
                    # attn @ v for head h
                    for r in range(R):
                        pav = pb2.tile([D, NQ], F32, tag="pav")
                        for jc in range(JC):
                            vt = v2.tile([P, D], F32R, tag="vt")
                            nc.sync.dma_start(vt[:], vM_d[:, r * JC + jc, h * D : (h + 1) * D])
                            nc.tensor.matmul(
                                pav[:], vt[:].bitcast(F32R), PT[:, jc, :].bitcast(F32R),
                                start=(jc == 0), stop=False)
                        nc.tensor.matmul(
                            pav[:], nullv_s[:, :].bitcast(F32R), PnT[:, :].bitcast(F32R),
                            start=False, stop=True)
                        avs = s2.tile([D, NQ], F32R, tag="avstage")
                        nc.any.tensor_copy(avs[:], pav[:])
                        nc.sync.dma_start(
                            aoT_d[pb : pb + D, g, r * NQ : (r + 1) * NQ], avs[:])

        # ---------------- Stage 3: out projection + final LN ------------
        with tc.tile_pool(name="w3", bufs=1) as w3, \
             tc.tile_pool(name="s3", bufs=8) as s3, \
             tc.tile_pool(name="s3b", bufs=4) as s3b, \
             tc.tile_pool(name="st3", bufs=6) as st3, \
             tc.tile_pool(name="p3", bufs=4, space="PSUM") as p3:

            wo_s = w3.tile([P, 4, C], F32R)
            for ec in range(4):
                nc.sync.dma_start(wo_s[:, ec, :], woT[ec * P : (ec + 1) * P, :])

            for rc in range(XCOLS // P):  # 48 row chunks
                pf = p3.tile([P, C], F32, tag="pf")
                for ec in range(4):
                    at = s3.tile([P, P], F32R, tag="at")
                    nc.sync.dma_start(at[:], aoT_d[:, ec, rc * P : (rc + 1) * P])
                    nc.tensor.matmul(
                        pf[:], at[:].bitcast(F32R), wo_s[:, ec, :].bitcast(F32R),
                        start=(ec == 0), stop=(ec == 3))
                nmean = st3.tile([P, 1], F32, tag="nmean")
                nc.vector.reduce_sum(nmean[:], pf[:], axis=AX)
                nc.scalar.mul(nmean[:], nmean[:], -1.0 / C)
                cen = s3b.tile([P, C], F32, tag="cen")
                nc.scalar.add(cen[:], pf[:], nmean[:])
                sq3 = s3b.tile([P, C], F32, tag="sq3")
                nc.scalar.activation(sq3[:], cen[:], AF.Square)
                var3 = st3.tile([P, 1], F32, tag="var3")
                nc.vector.reduce_sum(var3[:], sq3[:], axis=AX)
                nc.scalar.mul(var3[:], var3[:], 1.0 / C)
                std3 = st3.tile([P, 1], F32, tag="std3")
                nc.scalar.activation(std3[:], var3[:], AF.Sqrt, bias=eps_P[:])
                inv3 = st3.tile([P, 1], F32, tag="inv3")
                nc.vector.reciprocal(inv3[:], std3[:])
                on = s3b.tile([P, C], F32, tag="on")
                nc.vector.tensor_mul(on[:], cen[:], inv3[:].to_broadcast((P, C)))
                nc.vector.tensor_mul(on[:], on[:], outg_s[:, :])
                nc.sync.dma_start(out[rc * P : (rc + 1) * P, :], on[:])

    nc.compile()
    return nc


def kernel(x, context, norm_g, to_q_w, to_kv_w, null_kv, to_out_w, out_norm_g):
    from concourse.bass_utils import run_bass_kernel_spmd

    x = np.asarray(x, dtype=np.float32)
    context = np.asarray(context, dtype=np.float32)
    norm_g = np.asarray(norm_g, dtype=np.float32)
    to_q_w = np.asarray(to_q_w, dtype=np.float32)
    to_kv_w = np.asarray(to_kv_w, dtype=np.float32)
    null_kv = np.asarray(null_kv, dtype=np.float32)
    to_out_w = np.asarray(to_out_w, dtype=np.float32)
    out_norm_g = np.asarray(out_norm_g, dtype=np.float32)

    if "nc" not in _CACHE:
        _CACHE["nc"] = _build_program()
    nc = _CACHE["nc"]

    scale = (D ** -0.5) / ALPHA * (R ** -0.5)
    wq = np.ascontiguousarray((to_q_w * norm_g[None, :] * scale).T)
    wk = np.ascontiguousarray(to_kv_w[:E].T)
    wv = np.ascontiguousarray(to_kv_w[E:].T)
    wo = np.ascontiguousarray(to_out_w.T)
    nullk_a = np.ascontiguousarray(np.repeat(null_kv[0].reshape(D, 1), 2, axis=1))
    nullv_a = np.ascontiguousarray(null_kv[1].reshape(1, D))
    outg_a = np.ascontiguousarray(out_norm_g.reshape(1, C))
    ones_a = np.ones((P, 1), dtype=np.float32)

    in_maps = []
    for core in range(8):
        bi, half = core // 2, core % 2
        xs = x[bi, half * NQ : (half + 1) * NQ]          # [512, 12, 512]
        xT_a = np.ascontiguousarray(xs.transpose(2, 1, 0).reshape(C, XCOLS))
        cs = context[bi]                                  # [1024, 12, 512]
        ctxT_a = np.ascontiguousarray(cs.transpose(2, 1, 0).reshape(C, CCOLS))
        in_maps.append(dict(
            xT=xT_a, ctxT=ctxT_a, wqT=wq, wkT=wk, wvT=wv, woT=wo,
            nullk=nullk_a, nullv=nullv_a, outg=outg_a, onesc=ones_a))

    trace = bool(int(os.environ.get("KERNEL_TRACE", "0")))
    res = run_bass_kernel_spmd(nc, in_maps, list(range(8)), trace=trace)
    _CACHE["last_exec_ns"] = res.exec_time_ns

    outs = []
    for core in range(8):
        o = res.results[core]["out"]                      # [6144, 512], rows (r, i)
        outs.append(o.reshape(R, NQ, C).transpose(1, 0, 2))  # [512, 12, 512]
    full = np.stack(
        [np.concatenate([outs[2 * bi], outs[2 * bi + 1]], axis=0) for bi in range(B)])
    return full.astype(np.float32)



# revision 44
# speedup vs baseline: 1.2701x; 1.2701x over previous
"""Trainium2 Bass kernel for nn_Attention_v2_cross (dense transformer, 8 cores).

Sharding: 8 cores = 4 batches x 2 query-halves (data parallel over batch and
query positions). Every core holds the full weights and the full context for
its batch; the kv projection is duplicated across the two cores sharing a
batch (+25% matmul flops, zero collectives).

V2 layout (all matmuls bf16 in / fp32 PSUM accumulate):
  - Everything off-chip is bf16; host pre-casts inputs, host post-casts the
    bf16 output and applies out_norm_g (per-column scale commutes with the
    unshard).
  - Transposed softmax with no row-max: pb-relax keeps |alpha*sim| ~ 1.5 so
    exp() is safe, and softmax is exactly shift-invariant, so skipping the
    detached max matches the reference. sim is computed as simT[j, i]
    (keys on partitions) which kills the P-transposes of the old layout;
    key-axis sums come from ones-vector matmuls.
  - K=128 packing: q/k staged in DRAM as [e, rpair, rparity, cols]; per-head
    remap DMAs load [rr*64+d] partition layouts so sim contracts K=128
    (two r per matmul) and attn@v emits M=128 (two r per output tile).
  - LN folded into the q projection: stats via ones-matmuls on xT, centering
    via a K=1 wqsum x (-mean) matmul accumulated into the projection PSUM,
    scaling by inv(std) in the single PSUM->SBUF pass (DVE).
  - Final LN: mean via a wosum matmul (exact), variance via activation
    Square with accum_out, normalize in the activation copy (per-partition
    scale), g applied on host.
"""

import os
import numpy as np

B, N, R, C = 4, 1024, 12, 512
H, D = 8, 64
E = H * D            # 512
NQ = N // 2          # 512 queries per core
NKJ = N              # 1024 keys per core
ALPHA = 128.0
EPS = 1e-5
XCOLS = R * NQ       # 6144  (col = r*NQ + i)
CCOLS = R * NKJ      # 12288 (col = r*NKJ + j)
P = 128
RP = R // 2          # 6 r-pairs
JC = NKJ // P        # 8 key chunks of 128

_CACHE = {}


def _build_program():
    from contextlib import ExitStack
    import concourse.bass as bass
    import concourse.tile as tile
    from concourse import bacc
    from concourse import mybir

    F32 = mybir.dt.float32
    BF16 = mybir.dt.bfloat16
    AF = mybir.ActivationFunctionType
    AX = mybir.AxisListType.X

    nc = bacc.Bacc("TRN2", target_bir_lowering=False, debug=False, num_devices=8)

    # host inputs (bf16 except where noted)
    xT = nc.dram_tensor("xT", [P, 4, XCOLS], BF16, kind="ExternalInput").ap()
    ctxT = nc.dram_tensor("ctxT", [P, 4, CCOLS], BF16, kind="ExternalInput").ap()
    wqT = nc.dram_tensor("wqT", [P, 4, E], BF16, kind="ExternalInput").ap()
    wkT = nc.dram_tensor("wkT", [P, 4, E], BF16, kind="ExternalInput").ap()
    wvT = nc.dram_tensor("wvT", [P, 4, E], BF16, kind="ExternalInput").ap()
    woT = nc.dram_tensor("woT", [P, 4, C], BF16, kind="ExternalInput").ap()
    wqsum = nc.dram_tensor("wqsum", [1, E], BF16, kind="ExternalInput").ap()
    wosum = nc.dram_tensor("wosum", [P, 4], BF16, kind="ExternalInput").ap()
    nullk2 = nc.dram_tensor("nullk2", [P, 1], BF16, kind="ExternalInput").ap()
    nullv2 = nc.dram_tensor("nullv2", [1, P], BF16, kind="ExternalInput").ap()
    onesc = nc.dram_tensor("onesc", [P, 1], BF16, kind="ExternalInput").ap()
    out = nc.dram_tensor("out", [XCOLS, C], BF16, kind="ExternalOutput").ap()

    with ExitStack() as ctx:
        tc = ctx.enter_context(tile.TileContext(nc))

        const = ctx.enter_context(tc.tile_pool(name="const", bufs=1))
        dram = ctx.enter_context(tc.tile_pool(name="dram", bufs=1, space="DRAM"))
        persist = ctx.enter_context(tc.tile_pool(name="persist", bufs=1))

        ones_col = const.tile([P, 1], BF16)
        nc.sync.dma_start(ones_col[:], onesc[:, :])
        ones_row = const.tile([1, P], BF16)
        nc.vector.memset(ones_row[:], 1.0)
        nullk_s = const.tile([P, 1], BF16)
        nc.sync.dma_start(nullk_s[:], nullk2[:, :])
        nullv_s = const.tile([1, P], BF16)
        nc.sync.dma_start(nullv_s[:], nullv2[:, :])
        wqsum_s = const.tile([1, 4, P], BF16)
        nc.sync.dma_start(wqsum_s[:], wqsum.rearrange("one (ec p) -> one ec p", p=P))
        wosum_s = const.tile([P, 4], BF16)
        nc.sync.dma_start(wosum_s[:], wosum[:, :])
        eps_1 = const.tile([1, 1], F32)
        nc.vector.memset(eps_1[:], EPS)
        eps_P = const.tile([P, 1], F32)
        nc.vector.memset(eps_P[:], EPS)

        # DRAM staging, layouts chosen so stage-2 head loads are plain slices:
        #   q_d/k_d: [p, ec, rpair, rparity, cols]  (e = ec*128 + p)
        q_d = dram.tile([P, 4, RP, 2, NQ], BF16)
        k_d = dram.tile([P, 4, RP, 2, NKJ], BF16)

        # attention output, SBUF resident: [p=(hh,d), ec, rpair, rparity, i]
        aoT_s = persist.tile([P, 4, RP, 2, NQ], BF16)
        # v, SBUF resident in attention layout, one tile per 4-head group:
        # [p=j mod 128, rpair, jc, hh4, rparity, d] so the attn@v stationary
        # slice [(rr,d)] is one contiguous 128-wide free dim
        v_g = [persist.tile([P, RP, JC, 4, 2, D], BF16, name=f"v_g{i}")
               for i in range(2)]

        # ---------------- Stage 1: projections -------------------------
        with tc.tile_pool(name="w1", bufs=1) as wpool, \
             tc.tile_pool(name="s1", bufs=2) as s1, \
             tc.tile_pool(name="s1b", bufs=2) as s1b, \
             tc.tile_pool(name="st1", bufs=1) as st1, \
             tc.tile_pool(name="p1", bufs=4, space="PSUM") as p1, \
             tc.tile_pool(name="p1s", bufs=1, space="PSUM") as p1s:

            wq_s = wpool.tile([P, 4, E], BF16)
            wk_s = wpool.tile([P, 4, E], BF16)
            wv_s = wpool.tile([P, 4, E], BF16)
            nc.sync.dma_start(wq_s[:], wqT[:, :, :])
            nc.sync.dma_start(wk_s[:], wkT[:, :, :])
            nc.sync.dma_start(wv_s[:], wvT[:, :, :])

            # ---- 1a: LN(x) folded into q projection (transposed out) ----
            for rb in range(R):
                xt = s1.tile([P, 4, NQ], BF16, tag="xt")
                nc.sync.dma_start(xt[:], xT[:, :, rb * NQ : (rb + 1) * NQ])
                psum_sum = p1s.tile([1, NQ], F32, tag="stat_sum")
                psum_sq = p1s.tile([1, NQ], F32, tag="stat_sq")
                for cc in range(4):
                    nc.tensor.matmul(psum_sum[:], ones_col[:], xt[:, cc, :],
                                     start=(cc == 0), stop=(cc == 3))
                sq = s1.tile([P, 4, NQ], BF16, tag="sq", bufs=1)
                for cc in range(4):
                    # split the squares between Act and Pool
                    if cc < 2:
                        nc.scalar.activation(sq[:, cc, :], xt[:, cc, :], AF.Square)
                    else:
                        nc.gpsimd.tensor_tensor(
                            sq[:, cc, :], xt[:, cc, :], xt[:, cc, :],
                            mybir.AluOpType.mult)
                for cc in range(4):
                    nc.tensor.matmul(psum_sq[:], ones_col[:], sq[:, cc, :],
                                     start=(cc == 0), stop=(cc == 3))
                mean = st1.tile([1, NQ], F32, tag="mean")
                nc.vector.tensor_scalar_mul(mean[:], psum_sum[:], 1.0 / C)
                msq = st1.tile([1, NQ], F32, tag="msq")
                nc.gpsimd.tensor_tensor(msq[:], mean[:], mean[:],
                                        mybir.AluOpType.mult)
                var = st1.tile([1, NQ], F32, tag="var")
                nc.vector.tensor_scalar_mul(var[:], psum_sq[:], 1.0 / C)
                nc.gpsimd.tensor_tensor(var[:], var[:], msq[:],
                                        mybir.AluOpType.subtract)
                negmu = st1.tile([1, NQ], BF16, tag="negmu")
                nc.gpsimd.tensor_scalar_mul(negmu[:], mean[:], -1.0)
                std = mean  # mean is dead once negmu is out; reuse the tile
                nc.scalar.activation(std[:], var[:], AF.Sqrt, bias=eps_1[:])
                inv_bf = st1.tile([1, NQ], BF16, tag="inv")
                with nc.allow_low_precision(reason="bf16 LN scale, tol 2e-2"):
                    nc.vector.reciprocal(inv_bf[:], std[:])
                # broadcast inv across partitions via K=1 matmul
                inv_b = p1s.tile([P, NQ], F32, tag="inv_b")
                nc.tensor.matmul(inv_b[:], ones_row[:], inv_bf[:],
                                 start=True, stop=True)
                inv_s = s1.tile([P, NQ], BF16, tag="inv_s", bufs=1)
                nc.scalar.copy(inv_s[:], inv_b[:])
                qs = s1b.tile([P, 4, NQ], BF16, tag="qstage")
                for ec in range(4):
                    pq = p1.tile([P, NQ], F32, tag="proj")
                    # centering: accumulate wqsum (x) (-mean) into the PSUM
                    nc.tensor.matmul(pq[:], wqsum_s[:, ec, :], negmu[:],
                                     start=True, stop=False)
                    for cc in range(4):
                        nc.tensor.matmul(
                            pq[:], wq_s[:, cc, ec * P : (ec + 1) * P], xt[:, cc, :],
                            start=False, stop=(cc == 3))
                    # scale by inv(std) during the PSUM->SBUF pass
                    nc.vector.tensor_mul(qs[:, ec, :], pq[:], inv_s[:])
                nc.sync.dma_start(
                    q_d[:, :, rb // 2, rb % 2, :], qs[:])

            # ---- 1b: k projection (transposed) + v projection (row-major) ----
            for cb in range(CCOLS // NQ):  # 24 blocks of 512 context columns
                r, jb = cb // 2, cb % 2
                ct = s1.tile([P, 4, NQ], BF16, tag="ct")
                nc.sync.dma_start(ct[:], ctxT[:, :, cb * NQ : (cb + 1) * NQ])
                ks = s1b.tile([P, 4, NQ], BF16, tag="kstage")
                for ec in range(4):
                    pk = p1.tile([P, NQ], F32, tag="proj")
                    for cc in range(4):
                        nc.tensor.matmul(
                            pk[:], wk_s[:, cc, ec * P : (ec + 1) * P], ct[:, cc, :],
                            start=(cc == 0), stop=(cc == 3))
                    if ec % 2 == 0:
                        nc.scalar.copy(ks[:, ec, :], pk[:])
                    else:
                        nc.vector.tensor_copy(ks[:, ec, :], pk[:])
                nc.sync.dma_start(
                    k_d[:, :, r // 2, r % 2, jb * NQ : (jb + 1) * NQ], ks[:])
                for rc4 in range(4):
                    pv = p1.tile([P, E], F32, tag="proj")
                    for cc in range(4):
                        nc.tensor.matmul(
                            pv[:], ct[:, cc, rc4 * P : (rc4 + 1) * P], wv_s[:, cc, :],
                            start=(cc == 0), stop=(cc == 3))
                    jc = jb * 4 + rc4
                    # scatter straight into the SBUF-resident attention layout
                    nc.scalar.copy(
                        v_g[0][:, r // 2, jc, :, r % 2, :],
                        pv[:, 0 : 4 * D].rearrange("p (h d) -> p h d", d=D))
                    nc.vector.tensor_copy(
                        v_g[1][:, r // 2, jc, :, r % 2, :],
                        pv[:, 4 * D : E].rearrange("p (h d) -> p h d", d=D))

        stages = os.environ.get("KERNEL_STAGES", "123")
        if "2" not in stages:
            nc.sync.dma_start(
                out.rearrange("(p q) i -> p q i", p=P),
                q_d[:, :, :, :, :])

        # ---------------- Stage 2: attention ---------------------------
        with tc.tile_pool(name="qk2", bufs=2) as qk2, \
             tc.tile_pool(name="e2", bufs=2) as e2, \
             tc.tile_pool(name="o2", bufs=2) as o2, \
             tc.tile_pool(name="st2", bufs=2) as st2, \
             tc.tile_pool(name="pa2", bufs=3, space="PSUM") as pa2, \
             tc.tile_pool(name="pb2", bufs=2, space="PSUM") as pb2, \
             tc.tile_pool(name="ps2", bufs=1, space="PSUM") as ps2:

            for h in range(H if "2" in stages else 0):
                g, hh = h // 2, h % 2
                q_att = qk2.tile([P, RP, NQ], BF16, tag="q_att")
                k_att = qk2.tile([P, RP, NKJ], BF16, tag="k_att")
                for rr in range(2):
                    nc.sync.dma_start(
                        q_att[rr * D : (rr + 1) * D, :, :],
                        q_d[hh * D : (hh + 1) * D, g, :, rr, :])
                    nc.sync.dma_start(
                        k_att[rr * D : (rr + 1) * D, :, :],
                        k_d[hh * D : (hh + 1) * D, g, :, rr, :])
                v_att = v_g[h // 4]
                hh4 = h % 4

                # null-key sim: contract (rr, d) x nullk per r-pair
                pn = ps2.tile([1, NQ], F32, tag="pn")
                for rp in range(RP):
                    nc.tensor.matmul(pn[:], nullk_s[:], q_att[:, rp, :],
                                     start=(rp == 0), stop=(rp == RP - 1))
                en = st2.tile([1, NQ], BF16, tag="en")
                nc.scalar.activation(en[:], pn[:], AF.Exp, scale=ALPHA)

                exp_s = e2.tile([P, JC, NQ], BF16, tag="exp", bufs=1)
                for jc in range(JC):
                    psim = pa2.tile([P, NQ], F32, tag="sim")
                    for rp in range(RP):
                        nc.tensor.matmul(
                            psim[:],
                            k_att[:, rp, jc * P : (jc + 1) * P],
                            q_att[:, rp, :],
                            start=(rp == 0), stop=(rp == RP - 1))
                    nc.scalar.activation(exp_s[:, jc, :], psim[:], AF.Exp,
                                         scale=ALPHA)
                # key-axis sums via ones matmuls, + null column
                psums = ps2.tile([1, NQ], F32, tag="psums")
                for jc in range(JC):
                    nc.tensor.matmul(psums[:], ones_col[:], exp_s[:, jc, :],
                                     start=(jc == 0), stop=(jc == JC - 1))
                sums = st2.tile([1, NQ], F32, tag="sums")
                nc.vector.tensor_add(sums[:], psums[:], en[:])
                rinv = st2.tile([1, NQ], BF16, tag="rinv")
                with nc.allow_low_precision(reason="softmax scale, killed by out LN"):
                    nc.vector.reciprocal(rinv[:], sums[:])
                rinv_b = ps2.tile([P, NQ], F32, tag="rinv_b")
                nc.tensor.matmul(rinv_b[:], ones_row[:], rinv[:],
                                 start=True, stop=True)
                rinv_s = st2.tile([P, NQ], BF16, tag="rinv_s", bufs=1)
                nc.scalar.copy(rinv_s[:], rinv_b[:])

                out2 = o2.tile([P, RP, NQ], BF16, tag="out2", bufs=1)
                for rp in range(RP):
                    pav = pb2.tile([P, NQ], F32, tag="pav")
                    # null-value contribution (same for every r)
                    nc.tensor.matmul(pav[:], nullv_s[:], en[:],
                                     start=True, stop=False)
                    for jc in range(JC):
                        nc.tensor.matmul(
                            pav[:], v_att[:, rp, jc, hh4, :, :],
                            exp_s[:, jc, :],
                            start=False, stop=(jc == JC - 1))
                    nc.vector.tensor_mul(out2[:, rp, :], pav[:], rinv_s[:])
                # partition-shift restage into out-projection layout
                for rr in range(2):
                    nc.sync.dma_start(
                        aoT_s[hh * D : (hh + 1) * D, g, :, rr, :],
                        out2[rr * D : (rr + 1) * D, :, :])

        if "3" not in stages and "2" in stages:
            nc.sync.dma_start(
                out.rearrange("(p q) i -> p q i", p=P),
                aoT_s[:])

        # ---------------- Stage 3: out projection + final LN ------------
        with tc.tile_pool(name="w3", bufs=1) as w3, \
             tc.tile_pool(name="s3", bufs=3) as s3, \
             tc.tile_pool(name="st3", bufs=4) as st3, \
             tc.tile_pool(name="p3", bufs=3, space="PSUM") as p3, \
             tc.tile_pool(name="p3s", bufs=2, space="PSUM") as p3s:

            if "3" in stages:
                wo_s = w3.tile([P, 4, C], BF16)
                nc.sync.dma_start(wo_s[:], woT[:, :, :])

            for rc in range(XCOLS // P if "3" in stages else 0):  # 48 row chunks
                r, ib = rc // 4, rc % 4
                pf = p3.tile([P, C], F32, tag="pf")
                pmu = p3s.tile([P, 1], F32, tag="pmu")
                for ec in range(4):
                    at = aoT_s[:, ec, r // 2, r % 2, ib * P : (ib + 1) * P]
                    nc.tensor.matmul(pf[:], at, wo_s[:, ec, :],
                                     start=(ec == 0), stop=(ec == 3))
                    nc.tensor.matmul(pmu[:], at, wosum_s[:, ec : ec + 1],
                                     start=(ec == 0), stop=(ec == 3))
                negmu = st3.tile([P, 1], F32, tag="negmu")
                nc.scalar.mul(negmu[:], pmu[:], -1.0 / C)
                cen = s3.tile([P, C], F32, tag="cen")
                nc.scalar.add(cen[:], pf[:], negmu[:])
                sq3 = s3.tile([P, C], BF16, tag="sq3")
                nc.gpsimd.tensor_tensor(sq3[:], cen[:], cen[:],
                                        mybir.AluOpType.mult)
                ssq = st3.tile([P, 1], F32, tag="ssq")
                nc.vector.reduce_sum(ssq[:], sq3[:], axis=AX)
                std3 = st3.tile([P, 1], F32, tag="std3")
                nc.scalar.activation(std3[:], ssq[:], AF.Sqrt, scale=1.0 / C,
                                     bias=eps_P[:])
                inv3 = st3.tile([P, 1], F32, tag="inv3")
                nc.vector.reciprocal(inv3[:], std3[:])
                on = s3.tile([P, C], BF16, tag="on")
                nc.vector.tensor_mul(on[:], cen[:], inv3[:].to_broadcast((P, C)))
                nc.sync.dma_start(out[rc * P : (rc + 1) * P, :], on[:])

    nc.compile()
    return nc


def kernel(x, context, norm_g, to_q_w, to_kv_w, null_kv, to_out_w, out_norm_g):
    import ml_dtypes
    from concourse.bass_utils import run_bass_kernel_spmd

    BF = ml_dtypes.bfloat16
    x = np.asarray(x, dtype=np.float32)
    context = np.asarray(context, dtype=np.float32)
    norm_g = np.asarray(norm_g, dtype=np.float32)
    to_q_w = np.asarray(to_q_w, dtype=np.float32)
    to_kv_w = np.asarray(to_kv_w, dtype=np.float32)
    null_kv = np.asarray(null_kv, dtype=np.float32)
    to_out_w = np.asarray(to_out_w, dtype=np.float32)
    out_norm_g = np.asarray(out_norm_g, dtype=np.float32)

    if "nc" not in _CACHE:
        _CACHE["nc"] = _build_program()
    nc = _CACHE["nc"]

    scale = (D ** -0.5) / ALPHA * (R ** -0.5)
    wq = (to_q_w * norm_g[None, :] * scale).T          # [c, e]
    wk = to_kv_w[:E].T
    wv = to_kv_w[E:].T
    wo = to_out_w.T                                    # [e, o]

    def split_p(w):  # [c, e] -> [P, 4, e] with c = cc*128 + p -> [p, cc, e]
        return np.ascontiguousarray(
            w.reshape(4, P, -1).transpose(1, 0, 2).astype(BF))

    wq_a, wk_a, wv_a, wo_a = split_p(wq), split_p(wk), split_p(wv), split_p(wo)
    wqsum_a = np.ascontiguousarray(wq.sum(axis=0).reshape(1, E).astype(BF))
    wosum_a = np.ascontiguousarray(
        wo.sum(axis=1).reshape(4, P).T.astype(BF))     # [p, ec], e = ec*128+p
    nullk_a = np.ascontiguousarray(np.tile(null_kv[0], 2).reshape(P, 1).astype(BF))
    nullv_a = np.ascontiguousarray(np.tile(null_kv[1], 2).reshape(1, P).astype(BF))
    ones_a = np.ones((P, 1), dtype=BF)

    in_maps = []
    for core in range(8):
        bi, half = core // 2, core % 2
        xs = x[bi, half * NQ : (half + 1) * NQ]          # [512, 12, 512]
        xT_a = np.ascontiguousarray(
            xs.transpose(2, 1, 0).reshape(4, P, XCOLS).transpose(1, 0, 2)
            .astype(BF))
        cs = context[bi]                                  # [1024, 12, 512]
        ctxT_a = np.ascontiguousarray(
            cs.transpose(2, 1, 0).reshape(4, P, CCOLS).transpose(1, 0, 2)
            .astype(BF))
        in_maps.append(dict(
            xT=xT_a, ctxT=ctxT_a, wqT=wq_a, wkT=wk_a, wvT=wv_a, woT=wo_a,
            wqsum=wqsum_a, wosum=wosum_a, nullk2=nullk_a, nullv2=nullv_a,
            onesc=ones_a))

    trace = bool(int(os.environ.get("KERNEL_TRACE", "0")))
    res = run_bass_kernel_spmd(nc, in_maps, list(range(8)), trace=trace)
    _CACHE["last_exec_ns"] = res.exec_time_ns

    outs = []
    for core in range(8):
        o = np.asarray(res.results[core]["out"], dtype=np.float32)
        outs.append(o.reshape(R, NQ, C).transpose(1, 0, 2))  # [512, 12, 512]
    full = np.stack(
        [np.concatenate([outs[2 * bi], outs[2 * bi + 1]], axis=0) for bi in range(B)])
    return (full * out_norm_g[None, None, None, :]).astype(np.float32)


# revision 56
# speedup vs baseline: 1.3311x; 1.0480x over previous
"""Trainium2 Bass kernel for nn_Attention_v2_cross (dense transformer, 8 cores).

Sharding: 8 cores = 4 batches x 2 query-halves (data parallel over batch and
query positions). Every core holds the full weights and the full context for
its batch; the kv projection is duplicated across the two cores sharing a
batch (+25% matmul flops, zero collectives).

V3: fp8e4m3 DoubleRow matmuls (K=256 per pass) for the q/k/v projections and
the whole attention stage; fp32 PSUM accumulate everywhere.
  - Weights are scaled x32 on the host so fp8 stays out of subnormals; the
    1/32^2 for sim folds into the exp() scale and the x32 on the v path is
    removed by the final layernorm (scale-invariant; eps is negligible).
  - Transposed softmax with no row-max: pb-relax keeps |exponent| ~< 1 so
    exp() is safe, and softmax is shift-invariant so skipping the detached
    max matches the reference. Keys live on partitions, so the old
    P-transposes disappear; key-axis sums come from ones-vector matmuls.
  - K/M=128 packing via r-pair layouts: q/k staged in DRAM as
    [e, ec, rpair, rparity, cols] (fp8), reloaded per head with a partition
    remap; v never leaves SBUF (the projection scatters straight into the
    attention layout).
  - LN folded into the q projection (stats via ones-matmuls, centering via a
    K=1 wqsum x (-mean) matmul into the PSUM, inv(std) applied in the single
    PSUM->SBUF pass). Final LN: mean via a wosum matmul, out_norm_g applied
    on the host (commutes with the unshard).
  - Output is bf16, upcast on host (out-projection and final LN stay bf16/f32
    - fp8 there would cost ~4% error; here it is ~0.5% total).
"""

import os
import numpy as np

B, N, R, C = 4, 1024, 12, 512
H, D = 8, 64
E = H * D            # 512
NQ = N // 2          # 512 queries per core
NKJ = N              # 1024 keys per core
ALPHA = 128.0
EPS = 1e-5
XCOLS = R * NQ       # 6144  (col = r*NQ + i)
CCOLS = R * NKJ      # 12288 (col = r*NKJ + j)
P = 128
RP = R // 2          # 6 r-pairs
JC = NKJ // P        # 8 key chunks of 128
WSCALE = 32.0        # fp8 weight prescale (power of two)
# exp scale: alpha * (d^-0.5 / alpha / sqrt(r)) / WSCALE^2  (q and k carry x32)
EXPSCALE = (D ** -0.5) * (R ** -0.5) / (WSCALE * WSCALE)

_CACHE = {}


def _build_program():
    from contextlib import ExitStack
    import concourse.bass as bass
    import concourse.tile as tile
    from concourse import bacc
    from concourse import mybir

    F32 = mybir.dt.float32
    BF16 = mybir.dt.bfloat16
    F8 = mybir.dt.float8e4
    AF = mybir.ActivationFunctionType
    AX = mybir.AxisListType.X
    DR = mybir.MatmulPerfMode.DoubleRow

    nc = bacc.Bacc("TRN2", target_bir_lowering=False, debug=False, num_devices=8)

    xT = nc.dram_tensor("xT", [P, 4, XCOLS], BF16, kind="ExternalInput").ap()
    xT8 = nc.dram_tensor("xT8", [P, 4, XCOLS], F8, kind="ExternalInput").ap()
    ctxT = nc.dram_tensor("ctxT", [P, 4, CCOLS], BF16, kind="ExternalInput").ap()
    ctxT8 = nc.dram_tensor("ctxT8", [P, 4, CCOLS], F8, kind="ExternalInput").ap()
    wqT = nc.dram_tensor("wqT", [P, 4, E], F8, kind="ExternalInput").ap()
    wkT = nc.dram_tensor("wkT", [P, 4, E], F8, kind="ExternalInput").ap()
    wvT = nc.dram_tensor("wvT", [P, 4, E], BF16, kind="ExternalInput").ap()
    woT = nc.dram_tensor("woT", [P, 4, C], BF16, kind="ExternalInput").ap()
    wqsum = nc.dram_tensor("wqsum", [1, E], BF16, kind="ExternalInput").ap()
    wosum = nc.dram_tensor("wosum", [P, 4], BF16, kind="ExternalInput").ap()
    nullk2 = nc.dram_tensor("nullk2", [P, 1], F8, kind="ExternalInput").ap()
    nullv2 = nc.dram_tensor("nullv2", [1, P], BF16, kind="ExternalInput").ap()
    onesc = nc.dram_tensor("onesc", [P, 1], BF16, kind="ExternalInput").ap()
    ones8c = nc.dram_tensor("ones8c", [P, 1], F8, kind="ExternalInput").ap()
    out = nc.dram_tensor("out", [XCOLS, C], BF16, kind="ExternalOutput").ap()

    with ExitStack() as ctx:
        tc = ctx.enter_context(tile.TileContext(nc))

        const = ctx.enter_context(tc.tile_pool(name="const", bufs=1))
        dram = ctx.enter_context(tc.tile_pool(name="dram", bufs=1, space="DRAM"))
        persist = ctx.enter_context(tc.tile_pool(name="persist", bufs=1))

        ones_col = const.tile([P, 1], BF16)
        nc.sync.dma_start(ones_col[:], onesc[:, :])
        ones8 = const.tile([P, 1], F8)
        nc.sync.dma_start(ones8[:], ones8c[:, :])
        ones_row = const.tile([1, P], BF16)
        nc.vector.memset(ones_row[:], 1.0)
        nullk_s = const.tile([P, 1], F8)
        nc.sync.dma_start(nullk_s[:], nullk2[:, :])
        nullv_s = const.tile([1, P], BF16)
        nc.sync.dma_start(nullv_s[:], nullv2[:, :])
        wqsum_s = const.tile([1, 4, P], BF16)
        nc.sync.dma_start(wqsum_s[:], wqsum.rearrange("one (ec p) -> one ec p", p=P))
        wosum_s = const.tile([P, 4], BF16)
        nc.sync.dma_start(wosum_s[:], wosum[:, :])
        eps_1 = const.tile([1, 1], F32)
        nc.vector.memset(eps_1[:], EPS)
        eps_P = const.tile([P, 1], F32)
        nc.vector.memset(eps_P[:], EPS)

        # DRAM staging, layouts chosen so stage-2 head loads are plain slices:
        #   q_d/k_d: [p, ec, rpair, rparity, cols]  (e = ec*128 + p)
        q_d = dram.tile([P, 4, RP, 2, NQ], F8)
        k_d = dram.tile([P, 4, RP, 2, NKJ], F8)

        # attention output, SBUF resident: [p=(hh,d), ec, rpair, rparity, i]
        aoT_s = persist.tile([P, 4, RP, 2, NQ], BF16)
        # v, SBUF resident in attention layout, one tile per 4-head group:
        # [p=j mod 128, rpair, jc, hh4, rparity, d] so the attn@v stationary
        # slice [(rr,d)] is one contiguous 128-wide free dim
        v_g = [persist.tile([P, RP, JC, 4, 2, D], BF16, name=f"v_g{i}")
               for i in range(2)]

        # ---------------- Stage 1: projections -------------------------
        with tc.tile_pool(name="w1", bufs=1) as wpool, \
             tc.tile_pool(name="s1", bufs=2) as s1, \
             tc.tile_pool(name="s1b", bufs=2) as s1b, \
             tc.tile_pool(name="st1", bufs=2) as st1, \
             tc.tile_pool(name="p1", bufs=4, space="PSUM") as p1, \
             tc.tile_pool(name="p1s", bufs=1, space="PSUM") as p1s:

            wq_s = wpool.tile([P, 4, E], F8)
            wk_s = wpool.tile([P, 4, E], F8)
            wv_s = wpool.tile([P, 4, E], BF16)
            nc.sync.dma_start(wq_s[:], wqT[:, :, :])
            nc.sync.dma_start(wk_s[:], wkT[:, :, :])
            nc.sync.dma_start(wv_s[:], wvT[:, :, :])

            # ---- 1a: LN(x) folded into q projection (transposed out) ----
            for rb in range(R):
                xt = s1.tile([P, 4, NQ], BF16, tag="xt")
                nc.sync.dma_start(xt[:], xT[:, :, rb * NQ : (rb + 1) * NQ])
                xt8 = s1.tile([P, 4, NQ], F8, tag="xt8")
                nc.sync.dma_start(xt8[:], xT8[:, :, rb * NQ : (rb + 1) * NQ])
                psum_sum = p1s.tile([1, NQ], F32, tag="stat_sum")
                psum_sq = p1s.tile([1, NQ], F32, tag="stat_sq")
                for cc in range(4):
                    nc.tensor.matmul(psum_sum[:], ones_col[:], xt[:, cc, :],
                                     start=(cc == 0), stop=(cc == 3))
                sq = s1.tile([P, 4, NQ], BF16, tag="sq", bufs=1)
                for cc in range(4):
                    nc.gpsimd.tensor_tensor(
                        sq[:, cc, :], xt[:, cc, :], xt[:, cc, :],
                        mybir.AluOpType.mult)
                for cc in range(4):
                    nc.tensor.matmul(psum_sq[:], ones_col[:], sq[:, cc, :],
                                     start=(cc == 0), stop=(cc == 3))
                # -mean doubles as the centering rhs; PSUM-side smalls on Act,
                # SBUF-side smalls on Pool, keeping DVE free for the q scaling
                negmu = st1.tile([1, NQ], BF16, tag="negmu")
                nc.scalar.mul(negmu[:], psum_sum[:], -1.0 / C)
                msq = st1.tile([1, NQ], F32, tag="msq")
                nc.gpsimd.tensor_tensor(msq[:], negmu[:], negmu[:],
                                        mybir.AluOpType.mult)
                var = st1.tile([1, NQ], F32, tag="var")
                nc.scalar.mul(var[:], psum_sq[:], 1.0 / C)
                nc.gpsimd.tensor_tensor(var[:], var[:], msq[:],
                                        mybir.AluOpType.subtract)
                std = msq  # dead after the subtract; reuse
                nc.scalar.activation(std[:], var[:], AF.Sqrt, bias=eps_1[:])
                inv_bf = st1.tile([1, NQ], BF16, tag="inv")
                with nc.allow_low_precision(reason="bf16 LN scale, tol 2e-2"):
                    nc.vector.reciprocal(inv_bf[:], std[:])
                # broadcast inv across partitions via K=1 matmul
                inv_b = p1s.tile([P, NQ], F32, tag="inv_b")
                nc.tensor.matmul(inv_b[:], ones_row[:], inv_bf[:],
                                 start=True, stop=True)
                inv_s = s1.tile([P, NQ], BF16, tag="inv_s", bufs=1)
                nc.scalar.copy(inv_s[:], inv_b[:])
                qs = s1b.tile([P, 4, NQ], F8, tag="qstage")
                for ec in range(4):
                    pq = p1.tile([P, NQ], F32, tag="proj")
                    # centering: accumulate wqsum (x) (-mean) into the PSUM
                    nc.tensor.matmul(pq[:], wqsum_s[:, ec, :], negmu[:],
                                     start=True, stop=False)
                    for t in range(2):
                        nc.tensor.matmul(
                            pq[:],
                            wq_s[:, 2 * t : 2 * t + 2, ec * P : (ec + 1) * P],
                            xt8[:, 2 * t : 2 * t + 2, :],
                            start=False, stop=(t == 1), perf_mode=DR)
                    # scale by inv(std) during the PSUM->SBUF pass
                    with nc.allow_low_precision(reason="fp8 q, absorbed by softmax"):
                        nc.vector.tensor_mul(qs[:, ec, :], pq[:], inv_s[:])
                nc.sync.dma_start(
                    q_d[:, :, rb // 2, rb % 2, :], qs[:])

            # ---- 1b: k projection (transposed) + v projection -----------
            for cb in range(CCOLS // NQ):  # 24 blocks of 512 context columns
                r, jb = cb // 2, cb % 2
                ct = s1.tile([P, 4, NQ], BF16, tag="ct")
                nc.sync.dma_start(ct[:], ctxT[:, :, cb * NQ : (cb + 1) * NQ])
                ct8 = s1.tile([P, 4, NQ], F8, tag="ct8")
                nc.sync.dma_start(ct8[:], ctxT8[:, :, cb * NQ : (cb + 1) * NQ])
                ks = s1b.tile([P, 4, NQ], F8, tag="kstage")
                for ec in range(4):
                    pk = p1.tile([P, NQ], F32, tag="proj")
                    for t in range(2):
                        nc.tensor.matmul(
                            pk[:],
                            wk_s[:, 2 * t : 2 * t + 2, ec * P : (ec + 1) * P],
                            ct8[:, 2 * t : 2 * t + 2, :],
                            start=(t == 0), stop=(t == 1), perf_mode=DR)
                    if ec % 2 == 0:
                        nc.scalar.copy(ks[:, ec, :], pk[:])
                    else:
                        nc.vector.tensor_copy(ks[:, ec, :], pk[:])
                nc.sync.dma_start(
                    k_d[:, :, r // 2, r % 2, jb * NQ : (jb + 1) * NQ], ks[:])
                for rc4 in range(4):
                    pv = p1.tile([P, E], F32, tag="proj")
                    for cc in range(4):
                        nc.tensor.matmul(
                            pv[:], ct[:, cc, rc4 * P : (rc4 + 1) * P],
                            wv_s[:, cc, :],
                            start=(cc == 0), stop=(cc == 3))
                    jc = jb * 4 + rc4
                    # scatter straight into the SBUF-resident attention layout
                    nc.scalar.copy(
                        v_g[0][:, r // 2, jc, :, r % 2, :],
                        pv[:, 0 : 4 * D].rearrange("p (h d) -> p h d", d=D))
                    nc.vector.tensor_copy(
                        v_g[1][:, r // 2, jc, :, r % 2, :],
                        pv[:, 4 * D : E].rearrange("p (h d) -> p h d", d=D))

        stages = os.environ.get("KERNEL_STAGES", "123")
        if "2" not in stages:
            nc.sync.dma_start(
                out[0 : XCOLS // 2, :].rearrange("(p q) i -> p q i", p=P),
                q_d[:, :, :, :, :].bitcast(BF16))

        # ---------------- Stage 2: attention ---------------------------
        with tc.tile_pool(name="qk2", bufs=2) as qk2, \
             tc.tile_pool(name="e2", bufs=2) as e2, \
             tc.tile_pool(name="o2", bufs=2) as o2, \
             tc.tile_pool(name="st2", bufs=2) as st2, \
             tc.tile_pool(name="pa2", bufs=3, space="PSUM") as pa2, \
             tc.tile_pool(name="pb2", bufs=2, space="PSUM") as pb2, \
             tc.tile_pool(name="ps2", bufs=1, space="PSUM") as ps2:

            for h in range(H if "2" in stages else 0):
                g, hh = h // 2, h % 2
                q_att = qk2.tile([P, RP, NQ], F8, tag="q_att")
                k_att = qk2.tile([P, RP, NKJ], F8, tag="k_att")
                for rr in range(2):
                    nc.sync.dma_start(
                        q_att[rr * D : (rr + 1) * D, :, :],
                        q_d[hh * D : (hh + 1) * D, g, :, rr, :])
                    nc.sync.dma_start(
                        k_att[rr * D : (rr + 1) * D, :, :],
                        k_d[hh * D : (hh + 1) * D, g, :, rr, :])
                v_att = v_g[h // 4]
                hh4 = h % 4

                # null-key sim: contract (rr, d) x nullk per r-pair
                pn = ps2.tile([1, NQ], F32, tag="pn")
                for rp in range(RP):
                    nc.tensor.matmul(pn[:], nullk_s[:], q_att[:, rp, :],
                                     start=(rp == 0), stop=(rp == RP - 1))
                en = st2.tile([1, NQ], BF16, tag="en")
                nc.scalar.activation(en[:], pn[:], AF.Exp, scale=EXPSCALE)

                exp_s = e2.tile([P, JC, NQ], BF16, tag="exp")
                for jc in range(JC):
                    psim = pa2.tile([P, NQ], F32, tag="sim")
                    for t in range(3):
                        nc.tensor.matmul(
                            psim[:],
                            k_att[:, 2 * t : 2 * t + 2, jc * P : (jc + 1) * P],
                            q_att[:, 2 * t : 2 * t + 2, :],
                            start=(t == 0), stop=(t == 2), perf_mode=DR)
                    nc.scalar.activation(exp_s[:, jc, :], psim[:], AF.Exp,
                                         scale=EXPSCALE)
                # key-axis sums via ones matmuls, + null column
                psums = ps2.tile([1, NQ], F32, tag="psums")
                for jc in range(JC):
                    nc.tensor.matmul(psums[:], ones_col[:], exp_s[:, jc, :],
                                     start=(jc == 0), stop=(jc == JC - 1))
                sums = st2.tile([1, NQ], F32, tag="sums")
                nc.vector.tensor_add(sums[:], psums[:], en[:])
                rinv = st2.tile([1, NQ], BF16, tag="rinv")
                with nc.allow_low_precision(reason="softmax scale, killed by out LN"):
                    nc.vector.reciprocal(rinv[:], sums[:])
                rinv_b = ps2.tile([P, NQ], F32, tag="rinv_b")
                nc.tensor.matmul(rinv_b[:], ones_row[:], rinv[:],
                                 start=True, stop=True)
                rinv_s = st2.tile([P, NQ], BF16, tag="rinv_s", bufs=1)
                nc.scalar.copy(rinv_s[:], rinv_b[:])

                out2 = o2.tile([P, RP, NQ], BF16, tag="out2", bufs=2)
                for rp in range(RP):
                    pav = pb2.tile([P, NQ], F32, tag="pav")
                    # null-value contribution (same for every r)
                    nc.tensor.matmul(pav[:], nullv_s[:], en[:],
                                     start=True, stop=False)
                    for jc in range(JC):
                        nc.tensor.matmul(
                            pav[:], v_att[:, rp, jc, hh4, :, :],
                            exp_s[:, jc, :],
                            start=False, stop=(jc == JC - 1))
                    nc.vector.tensor_mul(out2[:, rp, :], pav[:], rinv_s[:])
                # partition-shift restage into out-projection layout
                for rr in range(2):
                    nc.sync.dma_start(
                        aoT_s[hh * D : (hh + 1) * D, g, :, rr, :],
                        out2[rr * D : (rr + 1) * D, :, :])

        if "3" not in stages and "2" in stages:
            nc.sync.dma_start(
                out.rearrange("(p q) i -> p q i", p=P),
                aoT_s[:])

        # ---------------- Stage 3: out projection + final LN ------------
        with tc.tile_pool(name="w3", bufs=1) as w3, \
             tc.tile_pool(name="s3", bufs=3) as s3, \
             tc.tile_pool(name="st3", bufs=4) as st3, \
             tc.tile_pool(name="p3", bufs=3, space="PSUM") as p3, \
             tc.tile_pool(name="p3s", bufs=2, space="PSUM") as p3s:

            if "3" in stages:
                wo_s = w3.tile([P, 4, C], BF16)
                nc.sync.dma_start(wo_s[:], woT[:, :, :])

            for rc in range(XCOLS // P if "3" in stages else 0):  # 48 chunks
                r, ib = rc // 4, rc % 4
                pf = p3.tile([P, C], F32, tag="pf")
                pmu = p3s.tile([P, 1], F32, tag="pmu")
                for ec in range(4):
                    at = aoT_s[:, ec, r // 2, r % 2, ib * P : (ib + 1) * P]
                    nc.tensor.matmul(pf[:], at, wo_s[:, ec, :],
                                     start=(ec == 0), stop=(ec == 3))
                    nc.tensor.matmul(pmu[:], at, wosum_s[:, ec : ec + 1],
                                     start=(ec == 0), stop=(ec == 3))
                negmu = st3.tile([P, 1], F32, tag="negmu")
                nc.vector.tensor_scalar_mul(negmu[:], pmu[:], -1.0 / C)
                cen = s3.tile([P, C], F32, tag="cen")
                nc.scalar.add(cen[:], pf[:], negmu[:])
                sq3 = s3.tile([P, C], BF16, tag="sq3")
                if rc % 2 == 0:
                    nc.gpsimd.tensor_tensor(sq3[:], cen[:], cen[:],
                                            mybir.AluOpType.mult)
                else:
                    nc.vector.tensor_mul(sq3[:], cen[:], cen[:])
                ssq = st3.tile([P, 1], F32, tag="ssq")
                nc.vector.reduce_sum(ssq[:], sq3[:], axis=AX)
                std3 = st3.tile([P, 1], F32, tag="std3")
                nc.scalar.activation(std3[:], ssq[:], AF.Sqrt, scale=1.0 / C,
                                     bias=eps_P[:])
                inv3 = st3.tile([P, 1], F32, tag="inv3")
                nc.vector.reciprocal(inv3[:], std3[:])
                on = s3.tile([P, C], BF16, tag="on")
                if rc % 2 == 1:
                    nc.vector.tensor_mul(on[:], cen[:],
                                         inv3[:].to_broadcast((P, C)))
                else:
                    nc.gpsimd.tensor_tensor(on[:], cen[:],
                                            inv3[:].to_broadcast((P, C)),
                                            mybir.AluOpType.mult)
                nc.sync.dma_start(out[rc * P : (rc + 1) * P, :], on[:])

    nc.compile()
    return nc


def _host_feeds(x, context, norm_g, to_q_w, to_kv_w, null_kv, to_out_w):
    import ml_dtypes

    BF = ml_dtypes.bfloat16
    F8 = ml_dtypes.float8_e4m3

    wq = (to_q_w * norm_g[None, :] * WSCALE).T          # [c, e], x32 for fp8
    wk = to_kv_w[:E].T * WSCALE
    wv = to_kv_w[E:].T                                  # v path stays bf16
    wo = to_out_w.T                                     # [e, o]

    def split_p(w, dt):  # [c, e] -> [p, cc, e] with c = cc*128 + p
        return np.ascontiguousarray(
            w.reshape(4, P, -1).transpose(1, 0, 2).astype(dt))

    common = dict(
        wqT=split_p(wq, F8), wkT=split_p(wk, F8), wvT=split_p(wv, BF),
        woT=split_p(wo, BF),
        wqsum=np.ascontiguousarray(
            wq.astype(F8).astype(np.float32).sum(axis=0).reshape(1, E).astype(BF)),
        wosum=np.ascontiguousarray(wo.sum(axis=1).reshape(4, P).T.astype(BF)),
        nullk2=np.ascontiguousarray(
            np.tile(null_kv[0] * WSCALE, 2).reshape(P, 1).astype(F8)),
        nullv2=np.ascontiguousarray(
            np.tile(null_kv[1], 2).reshape(1, P).astype(BF)),
        onesc=np.ones((P, 1), dtype=BF),
        ones8c=np.ones((P, 1), dtype=F8),
    )

    def colmajor(a, dt):  # [rows, r, c] -> [p, cc, cols]
        return np.ascontiguousarray(
            a.transpose(2, 1, 0).reshape(4, P, -1).transpose(1, 0, 2).astype(dt))

    in_maps = []
    for core in range(8):
        bi, half = core // 2, core % 2
        xs = x[bi, half * NQ : (half + 1) * NQ]          # [512, 12, 512]
        cs = context[bi]                                  # [1024, 12, 512]
        m = dict(common)
        m["xT"] = colmajor(xs, BF)
        m["xT8"] = colmajor(xs, F8)
        m["ctxT"] = colmajor(cs, BF)
        m["ctxT8"] = colmajor(cs, F8)
        in_maps.append(m)
    return in_maps


def kernel(x, context, norm_g, to_q_w, to_kv_w, null_kv, to_out_w, out_norm_g):
    from concourse.bass_utils import run_bass_kernel_spmd

    x = np.asarray(x, dtype=np.float32)
    context = np.asarray(context, dtype=np.float32)
    norm_g = np.asarray(norm_g, dtype=np.float32)
    to_q_w = np.asarray(to_q_w, dtype=np.float32)
    to_kv_w = np.asarray(to_kv_w, dtype=np.float32)
    null_kv = np.asarray(null_kv, dtype=np.float32)
    to_out_w = np.asarray(to_out_w, dtype=np.float32)
    out_norm_g = np.asarray(out_norm_g, dtype=np.float32)

    if "nc" not in _CACHE:
        _CACHE["nc"] = _build_program()
    nc = _CACHE["nc"]

    in_maps = _host_feeds(x, context, norm_g, to_q_w, to_kv_w, null_kv, to_out_w)

    trace = bool(int(os.environ.get("KERNEL_TRACE", "0")))
    res = run_bass_kernel_spmd(nc, in_maps, list(range(8)), trace=trace)
    _CACHE["last_exec_ns"] = res.exec_time_ns

    outs = []
    for core in range(8):
        o = np.asarray(res.results[core]["out"], dtype=np.float32)
        outs.append(o.reshape(R, NQ, C).transpose(1, 0, 2))  # [512, 12, 512]
    full = np.stack(
        [np.concatenate([outs[2 * bi], outs[2 * bi + 1]], axis=0) for bi in range(B)])
    return (full * out_norm_g[None, None, None, :]).astype(np.float32)


# revision 57
# speedup vs baseline: 1.9637x; 1.4753x over previous
"""Trainium2 Bass kernel for nn_Attention_v2_cross (dense transformer, 8 cores).

Sharding: 8 cores = 4 batches x 2 query-halves (data parallel over batch and
query positions). Every core holds the full weights and the full context for
its batch; the kv projection is duplicated across the two cores sharing a
batch (+25% matmul flops, zero collectives).

V3: fp8e4m3 DoubleRow matmuls (K=256 per pass) for the q/k/v projections and
the whole attention stage; fp32 PSUM accumulate everywhere.
  - Weights are scaled x32 on the host so fp8 stays out of subnormals; the
    1/32^2 for sim folds into the exp() scale and the x32 on the v path is
    removed by the final layernorm (scale-invariant; eps is negligible).
  - Transposed softmax with no row-max: pb-relax keeps |exponent| ~< 1 so
    exp() is safe, and softmax is shift-invariant so skipping the detached
    max matches the reference. Keys live on partitions, so the old
    P-transposes disappear; key-axis sums come from ones-vector matmuls.
  - K/M=128 packing via r-pair layouts: q/k staged in DRAM as
    [e, ec, rpair, rparity, cols] (fp8), reloaded per head with a partition
    remap; v never leaves SBUF (the projection scatters straight into the
    attention layout).
  - LN folded into the q projection (stats via ones-matmuls, centering via a
    K=1 wqsum x (-mean) matmul into the PSUM, inv(std) applied in the single
    PSUM->SBUF pass). Final LN: mean via a wosum matmul, out_norm_g applied
    on the host (commutes with the unshard).
  - Output is bf16, upcast on host (out-projection and final LN stay bf16/f32
    - fp8 there would cost ~4% error; here it is ~0.5% total).
"""

import os
import numpy as np

B, N, R, C = 4, 1024, 12, 512
H, D = 8, 64
E = H * D            # 512
NQ = N // 2          # 512 queries per core
NKJ = N              # 1024 keys per core
ALPHA = 128.0
EPS = 1e-5
XCOLS = R * NQ       # 6144  (col = r*NQ + i)
CCOLS = R * NKJ      # 12288 (col = r*NKJ + j)
P = 128
RP = R // 2          # 6 r-pairs
JC = NKJ // P        # 8 key chunks of 128
WSCALE = 8.0         # fp8 q/k staging prescale (power of two)
# exp scale: alpha * (d^-0.5 / alpha / sqrt(r)) / WSCALE^2  (q and k carry x32)
EXPSCALE = (D ** -0.5) * (R ** -0.5) / (WSCALE * WSCALE)

_CACHE = {}


def _build_program():
    from contextlib import ExitStack
    import concourse.bass as bass
    import concourse.tile as tile
    from concourse import bacc
    from concourse import mybir

    F32 = mybir.dt.float32
    BF16 = mybir.dt.bfloat16
    F8 = mybir.dt.float8e4
    AF = mybir.ActivationFunctionType
    AX = mybir.AxisListType.X
    DR = mybir.MatmulPerfMode.DoubleRow

    nc = bacc.Bacc("TRN2", target_bir_lowering=False, debug=False, num_devices=8)

    xT = nc.dram_tensor("xT", [P, 4, XCOLS], BF16, kind="ExternalInput").ap()
    ctxT = nc.dram_tensor("ctxT", [P, 4, CCOLS], BF16, kind="ExternalInput").ap()
    wqT = nc.dram_tensor("wqT", [P, 4, E], BF16, kind="ExternalInput").ap()
    wkT = nc.dram_tensor("wkT", [P, 4, E], BF16, kind="ExternalInput").ap()
    wvT = nc.dram_tensor("wvT", [P, 4, E], BF16, kind="ExternalInput").ap()
    woT = nc.dram_tensor("woT", [P, 4, C], BF16, kind="ExternalInput").ap()
    wqsum = nc.dram_tensor("wqsum", [1, E], BF16, kind="ExternalInput").ap()
    wosum = nc.dram_tensor("wosum", [P, 4], BF16, kind="ExternalInput").ap()
    nullk2 = nc.dram_tensor("nullk2", [P, 1], F8, kind="ExternalInput").ap()
    nullv2 = nc.dram_tensor("nullv2", [1, P], BF16, kind="ExternalInput").ap()
    onesc = nc.dram_tensor("onesc", [P, 1], BF16, kind="ExternalInput").ap()
    ones8c = nc.dram_tensor("ones8c", [P, 1], F8, kind="ExternalInput").ap()
    out = nc.dram_tensor("out", [XCOLS, C], BF16, kind="ExternalOutput").ap()

    with ExitStack() as ctx:
        tc = ctx.enter_context(tile.TileContext(nc))

        const = ctx.enter_context(tc.tile_pool(name="const", bufs=1))
        dram = ctx.enter_context(tc.tile_pool(name="dram", bufs=1, space="DRAM"))
        persist = ctx.enter_context(tc.tile_pool(name="persist", bufs=1))

        ones_col = const.tile([P, 1], BF16)
        nc.sync.dma_start(ones_col[:], onesc[:, :])
        ones8 = const.tile([P, 1], F8)
        nc.sync.dma_start(ones8[:], ones8c[:, :])
        ones_row = const.tile([1, P], BF16)
        nc.vector.memset(ones_row[:], 1.0)
        nullk_s = const.tile([P, 1], F8)
        nc.sync.dma_start(nullk_s[:], nullk2[:, :])
        nullv_s = const.tile([1, P], BF16)
        nc.sync.dma_start(nullv_s[:], nullv2[:, :])
        wqsum_s = const.tile([1, 4, P], BF16)
        nc.sync.dma_start(wqsum_s[:], wqsum.rearrange("one (ec p) -> one ec p", p=P))
        wosum_s = const.tile([P, 4], BF16)
        nc.sync.dma_start(wosum_s[:], wosum[:, :])
        eps_1 = const.tile([1, 1], F32)
        nc.vector.memset(eps_1[:], EPS)
        eps_P = const.tile([P, 1], F32)
        nc.vector.memset(eps_P[:], EPS)

        # DRAM staging, layouts chosen so stage-2 head loads are plain slices:
        #   q_d/k_d: [p, ec, rpair, rparity, cols]  (e = ec*128 + p)
        q_d = dram.tile([P, 4, RP, 2, NQ], F8)
        k_d = dram.tile([P, 4, RP, 2, NKJ], F8)

        # attention output, SBUF resident: [p=(hh,d), ec, rpair, rparity, i]
        aoT_s = persist.tile([P, 4, RP, 2, NQ], BF16)
        # v, SBUF resident in attention layout, one tile per 4-head group:
        # [p=j mod 128, rpair, jc, hh4, rparity, d] so the attn@v stationary
        # slice [(rr,d)] is one contiguous 128-wide free dim
        v_g = [persist.tile([P, RP, JC, 4, 2, D], BF16, name=f"v_g{i}")
               for i in range(2)]

        # ---------------- Stage 1: projections -------------------------
        with tc.tile_pool(name="w1", bufs=1) as wpool, \
             tc.tile_pool(name="s1", bufs=2) as s1, \
             tc.tile_pool(name="s1b", bufs=2) as s1b, \
             tc.tile_pool(name="st1", bufs=2) as st1, \
             tc.tile_pool(name="p1", bufs=4, space="PSUM") as p1, \
             tc.tile_pool(name="p1s", bufs=1, space="PSUM") as p1s:

            wq_s = wpool.tile([P, 4, E], BF16)
            wk_s = wpool.tile([P, 4, E], BF16)
            wv_s = wpool.tile([P, 4, E], BF16)
            nc.sync.dma_start(wq_s[:], wqT[:, :, :])
            nc.sync.dma_start(wk_s[:], wkT[:, :, :])
            nc.sync.dma_start(wv_s[:], wvT[:, :, :])

            # ---- 1a: LN(x) folded into q projection (transposed out) ----
            for rb in range(R):
                xt = s1.tile([P, 4, NQ], BF16, tag="xt")
                nc.sync.dma_start(xt[:], xT[:, :, rb * NQ : (rb + 1) * NQ])
                psum_sum = p1s.tile([1, NQ], F32, tag="stat_sum")
                psum_sq = p1s.tile([1, NQ], F32, tag="stat_sq")
                for cc in range(4):
                    nc.tensor.matmul(psum_sum[:], ones_col[:], xt[:, cc, :],
                                     start=(cc == 0), stop=(cc == 3))
                sq = s1.tile([P, 4, NQ], BF16, tag="sq", bufs=1)
                for cc in range(4):
                    nc.gpsimd.tensor_tensor(
                        sq[:, cc, :], xt[:, cc, :], xt[:, cc, :],
                        mybir.AluOpType.mult)
                for cc in range(4):
                    nc.tensor.matmul(psum_sq[:], ones_col[:], sq[:, cc, :],
                                     start=(cc == 0), stop=(cc == 3))
                # -mean doubles as the centering rhs; PSUM-side smalls on Act,
                # SBUF-side smalls on Pool, keeping DVE free for the q scaling
                negmu = st1.tile([1, NQ], BF16, tag="negmu")
                nc.scalar.mul(negmu[:], psum_sum[:], -1.0 / C)
                msq = st1.tile([1, NQ], F32, tag="msq")
                nc.gpsimd.tensor_tensor(msq[:], negmu[:], negmu[:],
                                        mybir.AluOpType.mult)
                var = st1.tile([1, NQ], F32, tag="var")
                nc.scalar.mul(var[:], psum_sq[:], 1.0 / C)
                nc.gpsimd.tensor_tensor(var[:], var[:], msq[:],
                                        mybir.AluOpType.subtract)
                std = msq  # dead after the subtract; reuse
                nc.scalar.activation(std[:], var[:], AF.Sqrt, bias=eps_1[:])
                inv_bf = st1.tile([1, NQ], BF16, tag="inv")
                with nc.allow_low_precision(reason="bf16 LN scale, tol 2e-2"):
                    nc.vector.reciprocal(inv_bf[:], std[:])
                # broadcast inv across partitions via K=1 matmul
                inv_b = p1s.tile([P, NQ], F32, tag="inv_b")
                nc.tensor.matmul(inv_b[:], ones_row[:], inv_bf[:],
                                 start=True, stop=True)
                inv_s = s1.tile([P, NQ], BF16, tag="inv_s", bufs=1)
                nc.scalar.copy(inv_s[:], inv_b[:])
                qs = s1b.tile([P, 4, NQ], F8, tag="qstage")
                for ec in range(4):
                    pq = p1.tile([P, NQ], F32, tag="proj")
                    # centering: accumulate wqsum (x) (-mean) into the PSUM
                    nc.tensor.matmul(pq[:], wqsum_s[:, ec, :], negmu[:],
                                     start=True, stop=False)
                    for cc in range(4):
                        nc.tensor.matmul(
                            pq[:], wq_s[:, cc, ec * P : (ec + 1) * P],
                            xt[:, cc, :],
                            start=False, stop=(cc == 3))
                    # scale by inv(std) during the PSUM->SBUF pass
                    with nc.allow_low_precision(reason="fp8 q, absorbed by softmax"):
                        nc.vector.tensor_mul(qs[:, ec, :], pq[:], inv_s[:])
                nc.sync.dma_start(
                    q_d[:, :, rb // 2, rb % 2, :], qs[:])

            # ---- 1b: k projection (transposed) + v projection -----------
            for cb in range(CCOLS // NQ):  # 24 blocks of 512 context columns
                r, jb = cb // 2, cb % 2
                ct = s1.tile([P, 4, NQ], BF16, tag="ct")
                nc.sync.dma_start(ct[:], ctxT[:, :, cb * NQ : (cb + 1) * NQ])
                ks = s1b.tile([P, 4, NQ], F8, tag="kstage")
                for ec in range(4):
                    pk = p1.tile([P, NQ], F32, tag="proj")
                    for cc in range(4):
                        nc.tensor.matmul(
                            pk[:], wk_s[:, cc, ec * P : (ec + 1) * P],
                            ct[:, cc, :],
                            start=(cc == 0), stop=(cc == 3))
                    if ec % 2 == 0:
                        nc.scalar.copy(ks[:, ec, :], pk[:])
                    else:
                        nc.vector.tensor_copy(ks[:, ec, :], pk[:])
                nc.sync.dma_start(
                    k_d[:, :, r // 2, r % 2, jb * NQ : (jb + 1) * NQ], ks[:])
                for rc4 in range(4):
                    pv = p1.tile([P, E], F32, tag="proj")
                    for cc in range(4):
                        nc.tensor.matmul(
                            pv[:], ct[:, cc, rc4 * P : (rc4 + 1) * P],
                            wv_s[:, cc, :],
                            start=(cc == 0), stop=(cc == 3))
                    jc = jb * 4 + rc4
                    # scatter straight into the SBUF-resident attention layout
                    nc.scalar.copy(
                        v_g[0][:, r // 2, jc, :, r % 2, :],
                        pv[:, 0 : 4 * D].rearrange("p (h d) -> p h d", d=D))
                    nc.vector.tensor_copy(
                        v_g[1][:, r // 2, jc, :, r % 2, :],
                        pv[:, 4 * D : E].rearrange("p (h d) -> p h d", d=D))

        stages = os.environ.get("KERNEL_STAGES", "123")
        if "2" not in stages:
            nc.sync.dma_start(
                out[0 : XCOLS // 2, :].rearrange("(p q) i -> p q i", p=P),
                q_d[:, :, :, :, :].bitcast(BF16))

        # ---------------- Stage 2: attention ---------------------------
        with tc.tile_pool(name="qk2", bufs=2) as qk2, \
             tc.tile_pool(name="e2", bufs=2) as e2, \
             tc.tile_pool(name="o2", bufs=2) as o2, \
             tc.tile_pool(name="st2", bufs=2) as st2, \
             tc.tile_pool(name="pa2", bufs=3, space="PSUM") as pa2, \
             tc.tile_pool(name="pb2", bufs=2, space="PSUM") as pb2, \
             tc.tile_pool(name="ps2", bufs=1, space="PSUM") as ps2:

            for h in range(H if "2" in stages else 0):
                g, hh = h // 2, h % 2
                q_att = qk2.tile([P, RP, NQ], F8, tag="q_att")
                k_att = qk2.tile([P, RP, NKJ], F8, tag="k_att")
                for rr in range(2):
                    nc.sync.dma_start(
                        q_att[rr * D : (rr + 1) * D, :, :],
                        q_d[hh * D : (hh + 1) * D, g, :, rr, :])
                    nc.sync.dma_start(
                        k_att[rr * D : (rr + 1) * D, :, :],
                        k_d[hh * D : (hh + 1) * D, g, :, rr, :])
                v_att = v_g[h // 4]
                hh4 = h % 4

                # null-key sim: contract (rr, d) x nullk per r-pair
                pn = ps2.tile([1, NQ], F32, tag="pn")
                for rp in range(RP):
                    nc.tensor.matmul(pn[:], nullk_s[:], q_att[:, rp, :],
                                     start=(rp == 0), stop=(rp == RP - 1))
                en = st2.tile([1, NQ], BF16, tag="en")
                nc.scalar.activation(en[:], pn[:], AF.Exp, scale=EXPSCALE)

                exp_s = e2.tile([P, JC, NQ], BF16, tag="exp")
                for jc in range(JC):
                    psim = pa2.tile([P, NQ], F32, tag="sim")
                    for t in range(3):
                        nc.tensor.matmul(
                            psim[:],
                            k_att[:, 2 * t : 2 * t + 2, jc * P : (jc + 1) * P],
                            q_att[:, 2 * t : 2 * t + 2, :],
                            start=(t == 0), stop=(t == 2), perf_mode=DR)
                    nc.scalar.activation(exp_s[:, jc, :], psim[:], AF.Exp,
                                         scale=EXPSCALE)
                # key-axis sums via ones matmuls, + null column
                psums = ps2.tile([1, NQ], F32, tag="psums")
                for jc in range(JC):
                    nc.tensor.matmul(psums[:], ones_col[:], exp_s[:, jc, :],
                                     start=(jc == 0), stop=(jc == JC - 1))
                sums = st2.tile([1, NQ], F32, tag="sums")
                nc.vector.tensor_add(sums[:], psums[:], en[:])
                rinv = st2.tile([1, NQ], BF16, tag="rinv")
                with nc.allow_low_precision(reason="softmax scale, killed by out LN"):
                    nc.vector.reciprocal(rinv[:], sums[:])
                rinv_b = ps2.tile([P, NQ], F32, tag="rinv_b")
                nc.tensor.matmul(rinv_b[:], ones_row[:], rinv[:],
                                 start=True, stop=True)
                rinv_s = st2.tile([P, NQ], BF16, tag="rinv_s", bufs=1)
                nc.scalar.copy(rinv_s[:], rinv_b[:])

                out2 = o2.tile([P, RP, NQ], BF16, tag="out2", bufs=2)
                for rp in range(RP):
                    pav = pb2.tile([P, NQ], F32, tag="pav")
                    # null-value contribution (same for every r)
                    nc.tensor.matmul(pav[:], nullv_s[:], en[:],
                                     start=True, stop=False)
                    for jc in range(JC):
                        nc.tensor.matmul(
                            pav[:], v_att[:, rp, jc, hh4, :, :],
                            exp_s[:, jc, :],
                            start=False, stop=(jc == JC - 1))
                    nc.vector.tensor_mul(out2[:, rp, :], pav[:], rinv_s[:])
                # partition-shift restage into out-projection layout
                for rr in range(2):
                    nc.sync.dma_start(
                        aoT_s[hh * D : (hh + 1) * D, g, :, rr, :],
                        out2[rr * D : (rr + 1) * D, :, :])

        if "3" not in stages and "2" in stages:
            nc.sync.dma_start(
                out.rearrange("(p q) i -> p q i", p=P),
                aoT_s[:])

        # ---------------- Stage 3: out projection + final LN ------------
        with tc.tile_pool(name="w3", bufs=1) as w3, \
             tc.tile_pool(name="s3", bufs=3) as s3, \
             tc.tile_pool(name="st3", bufs=4) as st3, \
             tc.tile_pool(name="p3", bufs=3, space="PSUM") as p3, \
             tc.tile_pool(name="p3s", bufs=2, space="PSUM") as p3s:

            if "3" in stages:
                wo_s = w3.tile([P, 4, C], BF16)
                nc.sync.dma_start(wo_s[:], woT[:, :, :])

            for rc in range(XCOLS // P if "3" in stages else 0):  # 48 chunks
                r, ib = rc // 4, rc % 4
                pf = p3.tile([P, C], F32, tag="pf")
                pmu = p3s.tile([P, 1], F32, tag="pmu")
                for ec in range(4):
                    at = aoT_s[:, ec, r // 2, r % 2, ib * P : (ib + 1) * P]
                    nc.tensor.matmul(pf[:], at, wo_s[:, ec, :],
                                     start=(ec == 0), stop=(ec == 3))
                    nc.tensor.matmul(pmu[:], at, wosum_s[:, ec : ec + 1],
                                     start=(ec == 0), stop=(ec == 3))
                negmu = st3.tile([P, 1], F32, tag="negmu")
                nc.vector.tensor_scalar_mul(negmu[:], pmu[:], -1.0 / C)
                cen = s3.tile([P, C], F32, tag="cen")
                nc.scalar.add(cen[:], pf[:], negmu[:])
                sq3 = s3.tile([P, C], BF16, tag="sq3")
                if rc % 2 == 0:
                    nc.gpsimd.tensor_tensor(sq3[:], cen[:], cen[:],
                                            mybir.AluOpType.mult)
                else:
                    nc.vector.tensor_mul(sq3[:], cen[:], cen[:])
                ssq = st3.tile([P, 1], F32, tag="ssq")
                nc.vector.reduce_sum(ssq[:], sq3[:], axis=AX)
                std3 = st3.tile([P, 1], F32, tag="std3")
                nc.scalar.activation(std3[:], ssq[:], AF.Sqrt, scale=1.0 / C,
                                     bias=eps_P[:])
                inv3 = st3.tile([P, 1], F32, tag="inv3")
                nc.vector.reciprocal(inv3[:], std3[:])
                on = s3.tile([P, C], BF16, tag="on")
                if rc % 2 == 1:
                    nc.vector.tensor_mul(on[:], cen[:],
                                         inv3[:].to_broadcast((P, C)))
                else:
                    nc.gpsimd.tensor_tensor(on[:], cen[:],
                                            inv3[:].to_broadcast((P, C)),
                                            mybir.AluOpType.mult)
                nc.sync.dma_start(out[rc * P : (rc + 1) * P, :], on[:])

    nc.compile()
    return nc


def _host_feeds(x, context, norm_g, to_q_w, to_kv_w, null_kv, to_out_w):
    import ml_dtypes

    BF = ml_dtypes.bfloat16
    F8 = ml_dtypes.float8_e4m3

    wq = (to_q_w * norm_g[None, :] * WSCALE).T          # [c, e], x32 for fp8
    wk = to_kv_w[:E].T * WSCALE
    wv = to_kv_w[E:].T                                  # v path stays bf16
    wo = to_out_w.T                                     # [e, o]

    def split_p(w, dt):  # [c, e] -> [p, cc, e] with c = cc*128 + p
        return np.ascontiguousarray(
            w.reshape(4, P, -1).transpose(1, 0, 2).astype(dt))

    common = dict(
        wqT=split_p(wq, BF), wkT=split_p(wk, BF), wvT=split_p(wv, BF),
        woT=split_p(wo, BF),
        wqsum=np.ascontiguousarray(
            wq.astype(BF).astype(np.float32).sum(axis=0).reshape(1, E).astype(BF)),
        wosum=np.ascontiguousarray(wo.sum(axis=1).reshape(4, P).T.astype(BF)),
        nullk2=np.ascontiguousarray(
            np.tile(null_kv[0] * WSCALE, 2).reshape(P, 1).astype(F8)),
        nullv2=np.ascontiguousarray(
            np.tile(null_kv[1], 2).reshape(1, P).astype(BF)),
        onesc=np.ones((P, 1), dtype=BF),
        ones8c=np.ones((P, 1), dtype=F8),
    )

    def colmajor(a, dt):  # [rows, r, c] -> [p, cc, cols]
        return np.ascontiguousarray(
            a.transpose(2, 1, 0).reshape(4, P, -1).transpose(1, 0, 2).astype(dt))

    in_maps = []
    for core in range(8):
        bi, half = core // 2, core % 2
        xs = x[bi, half * NQ : (half + 1) * NQ]          # [512, 12, 512]
        cs = context[bi]                                  # [1024, 12, 512]
        m = dict(common)
        m["xT"] = colmajor(xs, BF)
        m["ctxT"] = colmajor(cs, BF)
        in_maps.append(m)
    return in_maps


def kernel(x, context, norm_g, to_q_w, to_kv_w, null_kv, to_out_w, out_norm_g):
    from concourse.bass_utils import run_bass_kernel_spmd

    x = np.asarray(x, dtype=np.float32)
    context = np.asarray(context, dtype=np.float32)
    norm_g = np.asarray(norm_g, dtype=np.float32)
    to_q_w = np.asarray(to_q_w, dtype=np.float32)
    to_kv_w = np.asarray(to_kv_w, dtype=np.float32)
    null_kv = np.asarray(null_kv, dtype=np.float32)
    to_out_w = np.asarray(to_out_w, dtype=np.float32)
    out_norm_g = np.asarray(out_norm_g, dtype=np.float32)

    if "nc" not in _CACHE:
        _CACHE["nc"] = _build_program()
    nc = _CACHE["nc"]

    in_maps = _host_feeds(x, context, norm_g, to_q_w, to_kv_w, null_kv, to_out_w)

    trace = bool(int(os.environ.get("KERNEL_TRACE", "0")))
    res = run_bass_kernel_spmd(nc, in_maps, list(range(8)), trace=trace)
    _CACHE["last_exec_ns"] = res.exec_time_ns

    outs = []
    for core in range(8):
        o = np.asarray(res.results[core]["out"], dtype=np.float32)
        outs.append(o.reshape(R, NQ, C).transpose(1, 0, 2))  # [512, 12, 512]
    full = np.stack(
        [np.concatenate([outs[2 * bi], outs[2 * bi + 1]], axis=0) for bi in range(B)])
    return (full * out_norm_g[None, None, None, :]).astype(np.float32)
